# revision 83
# baseline (speedup 1.0000x reference)
"""Trainium2 Bass kernel for nn_MultiHeadAttention (B=4, S=2048, D=1024, H=16).

Sharding: 8 cores = 4 batches x 2 head-groups. Host ships x^T (bf16) so the
kernel never transposes on-chip. Each core: Q/K/V projections for its 512
columns, causal attention for its 8 heads, partial output projection
(row-parallel over wo); host sums the two partials per batch and adds bo.

v4 additions over v3:
  - fp8 (e4m3, DoubleRow 2-weights/cell) projections for everything that
    only feeds long-context queries: q/k/out-proj for query windows 1-3 and
    v for token tiles >= 4. Those paths' fp8 noise is suppressed ~1/sqrt(k)
    by softmax averaging over >= 512 keys; window 0 (short-context queries,
    no suppression) stays bf16 end to end. Weights pre-scaled x32 (and the
    fp8 out-proj staging x16) to sit in e4m3's normal range, divided back
    out in the EXP scale / PSUM drains. Cuts ~17us of PE streaming.
  - Output ships bf16 (host upcasts + sums partials in fp32).
  - Startup DMA order puts wv + x^T window 0 first; v tiles 0-7 run eagerly
    (4-7 via fp8) to cover the wq/wk DMA shadow; mid-window filler reserve
    (8 items at qc=1,2) guards the late windows against starvation; each
    pair's projections are force-pumped one attention-call ahead so their
    PSUM drain completes before the scores need them.
  - Softmax epilogue off the ACT/PE critical paths: ACT stages the sums
    rows to partition 0 (custom DVE ops ignore partition offsets), DVE
    reciprocal_approx_fast computes 1/sums, the otherwise-idle GpSimd
    broadcasts it across the head-dim partitions (partition_broadcast
    ucode), and one fused DVE scalar_tensor_tensor normalizes + applies the
    fp8 staging scale. Replaces the ACT ln/exp pair (-36us ACT) and the PE
    K=1 broadcast matmuls (-11us PE) of v3.

v3 design (all bf16 through the PE, fp32 PSUM):
  - Scores quadrant-packed: both heads of a pair computed concurrently in the
    128x128 PE array via 2x2 tile_position tiling with [64,64] stationaries
    (NumWeights=64 keeps FWL off - the bf16 base-64 128-col FWL path crashes).
    Halves score-matmul wall time.
  - q-window-outer (512 queries), pair-inner attention; one EXP instruction
    covers both heads' scores ([128, 2, w]).
  - Softmax 1/sums = exp(-ln(sums)) on ACT (ln+exp pinned to the
    natural_log_exp_and_others table set - avoids 32 table reloads);
    PE K=1 ones-matmul broadcasts across partitions; DVE multiply writes
    the bf16 staging tile. V's sums column is a memset constant.
  - Deficit-based filler pump: each attention iteration emits just enough
    projection/output-projection matmuls to cover the ACT exp latency, so
    the PE stream stays dense and the HAM clock gate stays at K=8/8.
"""

from collections import deque

import numpy as np

import concourse.bass as bass
import concourse.mybir as mybir
import concourse.tile as tile
from concourse import bacc
from concourse.masks import make_upper_triangular

F32 = mybir.dt.float32
BF16 = mybir.dt.bfloat16
FP8 = mybir.dt.float8e4
P = 128
AF = mybir.ActivationFunctionType
# q/k/v/output projections for token windows >= 1 run in fp8 (DoubleRow,
# 2x PE rate). Weights are pre-scaled by WSC so N(0, 1/32) values sit in
# e4m3's normal range; the factor is divided back out downstream (EXP scale
# for scores, the PSUM drain for v / out-proj). The attention-output staging
# for fp8 out-proj is pre-scaled by SSC so its ~N(0, 1/k_eff) values stay
# normal too. Window 0 (queries 0-511, which attend few keys and get no
# averaging suppression of the fp8 noise) stays bf16 end to end.
WSC = 32.0
SSC = 16.0


def _pin_act_tables(arch):
    """Mutate the cached activation-table map so Exp and Ln both resolve to
    natural_log_exp_and_others (set indices preserved -> one table load)."""
    try:
        from concourse.hw_specs import get_activation_tables
        tabs = get_activation_tables(arch)
    except Exception:
        return
    pin = tabs.get("natural_log_exp_and_others")
    if not pin or AF.Exp not in pin or AF.Ln not in pin:
        return
    for name, funcs in tabs.items():
        if name != "natural_log_exp_and_others":
            funcs.discard(AF.Exp)
            funcs.discard(AF.Ln)


def build_nc(S=2048, D=1024, HN=8, HD=64):
    MD = BF16
    C = HN * HD        # 512 local head-dims
    NT = S // P        # 16 token tiles
    ND = D // P        # 8 contraction tiles for projections
    NP = HN // 2       # 4 head pairs
    QW = 512           # query window (1 PSUM bank per head)
    NQC = S // QW      # 4
    VW = HD + 1
    SCALE = 1.0 / float(np.sqrt(HD))

    nc = bacc.Bacc("TRN2", target_bir_lowering=False)
    _pin_act_tables(nc.m.arch)

    xT_d = nc.dram_tensor("xT", [D, S], MD, kind="ExternalInput")
    wq_d = nc.dram_tensor("wq", [D, C], MD, kind="ExternalInput")
    wk_d = nc.dram_tensor("wk", [D, C], MD, kind="ExternalInput")
    # fp8 copies, host-packed for DoubleRow: x^T pre-arranged [P, ND, S];
    # weights [P, ND/2, 2, C] with wf[p, t, j, c] = WSC*w[256t + 128j + p, c].
    xf_d = nc.dram_tensor("xf", [P, D // P, S], FP8, kind="ExternalInput")
    wqf_d = nc.dram_tensor("wqf", [P, D // 256, 2, C], FP8, kind="ExternalInput")
    wkf_d = nc.dram_tensor("wkf", [P, D // 256, 2, C], FP8, kind="ExternalInput")
    wvf_d = nc.dram_tensor("wvf", [P, D // 256, 2, C], FP8, kind="ExternalInput")
    wof_d = nc.dram_tensor("wof", [P, C // 256, 2, D], FP8, kind="ExternalInput")
    wv_d = nc.dram_tensor("wv", [D, C], MD, kind="ExternalInput")
    wo_d = nc.dram_tensor("wo", [C, D], MD, kind="ExternalInput")
    bq_d = nc.dram_tensor("bq", [C], F32, kind="ExternalInput")
    bk_d = nc.dram_tensor("bk", [C], F32, kind="ExternalInput")
    bv_d = nc.dram_tensor("bv", [C], F32, kind="ExternalInput")
    # Output ships as bf16 (host upcasts and sums the two per-batch
    # partials in fp32): halves output DMA traffic; adds <=0.4% rounding
    # on top of a ~0.4% bf16 pipeline, well inside the error budget.
    out_d = nc.dram_tensor("out", [S, D], MD, kind="ExternalOutput")

    with tile.TileContext(nc) as tc:
        from contextlib import ExitStack

        with ExitStack() as ctx:
            singles = ctx.enter_context(tc.tile_pool(name="singles", bufs=1))
            # ut1[k, q] = 1.0 where k <= q (valid causal region of a diagonal
            # tile in S^T = [k, q] layout).
            ut1 = singles.tile([P, P], F32)
            make_upper_triangular(nc, ut1[:], val=1.0, diag=True)
            ones1 = singles.tile([1, HD], MD)
            nc.vector.memset(ones1[:], 1.0)
            bq_sb = singles.tile([P, NP], F32)
            bk_sb = singles.tile([P, NP], F32)
            bv_sb = singles.tile([P, C], F32)
            bq32_sb = singles.tile([P, NP], F32)
            bk32_sb = singles.tile([P, NP], F32)

            wq_pool = ctx.enter_context(tc.tile_pool(name="wq", bufs=1))
            wq_sb = wq_pool.tile([P, ND, C], MD)
            wk_sb = wq_pool.tile([P, ND, C], MD)
            wv_sb = wq_pool.tile([P, ND, C], MD)
            wo_sb = wq_pool.tile([P, NP, D], MD)
            wqf_sb = wq_pool.tile([P, ND // 2, 2, C], FP8)
            wkf_sb = wq_pool.tile([P, ND // 2, 2, C], FP8)
            wvf_sb = wq_pool.tile([P, ND // 2, 2, C], FP8)
            wof_sb = wq_pool.tile([P, NP // 2, 2, D], FP8)

            xT_pool = ctx.enter_context(tc.tile_pool(name="xT", bufs=1))
            # bf16 x^T only covers window 0 (the bf16 projection paths);
            # windows 1-3 are consumed exclusively through the fp8 copy.
            xT = xT_pool.tile([P, ND, QW], MD)
            xf = xT_pool.tile([P, ND, S], FP8)

            v_pool = ctx.enter_context(tc.tile_pool(name="v", bufs=1))
            v_sb = v_pool.tile([P, NT, HN, VW], MD)

            qkT_pool = ctx.enter_context(tc.tile_pool(name="qkT", bufs=1))
            qTp = [qkT_pool.tile([P, S], MD, name=f"qTp{p}") for p in range(NP)]
            kTp = [qkT_pool.tile([P, S], MD, name=f"kTp{p}") for p in range(NP)]

            stg_pool = ctx.enter_context(tc.tile_pool(name="stg", bufs=1))
            stg = stg_pool.tile([P, NQC, NP, QW], MD)
            stgf = stg_pool.tile([P, NQC, NP, QW], FP8)

            pT_pool = ctx.enter_context(tc.tile_pool(name="pT", bufs=4))
            ocp_pool = ctx.enter_context(tc.tile_pool(name="ocp", bufs=2))
            rc_pool = ctx.enter_context(tc.tile_pool(name="rc", bufs=2))
            bc_pool = ctx.enter_context(tc.tile_pool(name="bc", bufs=2))
            ost_pool = ctx.enter_context(tc.tile_pool(name="ost", bufs=4))

            # PSUM: scores 2 slots x [P,2,512] (2 banks each) + O^T 2 x 1 bank
            # + misc (projections / broadcast / out-proj) 2 x 1 bank = 8.
            s_pool = ctx.enter_context(tc.tile_pool(name="ps_s", bufs=2, space="PSUM"))
            o_pool = ctx.enter_context(tc.tile_pool(name="ps_o", bufs=1, space="PSUM"))
            m_pool = ctx.enter_context(tc.tile_pool(name="ps_m", bufs=2, space="PSUM"))

            # ---- DMA schedule (priority order) ---------------------------
            # Input DMAs split across the two HWDGE rings (sync + scalar) so
            # wv and the first x^T window transfer in parallel at startup.
            # Biases follow the first compute's inputs: they are only needed
            # by the PSUM drains, ~2us after the first matmul.
            xr = xT_d.rearrange("(o p) n -> p o n", p=P)
            wvr = wv_d.rearrange("(o p) n -> p o n", p=P)
            nc.scalar.dma_start(xT[:, :, 0:256], xr[:, :, 0:256])
            nc.sync.dma_start(wv_sb[:, 0:4, :], wvr[:, 0:4, :])
            nc.scalar.dma_start(xT[:, :, 256:QW], xr[:, :, 256:QW])
            nc.sync.dma_start(wv_sb[:, 4:8, :], wvr[:, 4:8, :])
            nc.sync.dma_start(
                bv_sb[:], bass.AP(tensor=bv_d, offset=0, ap=[[0, P], [1, C]])
            )
            # fp8 x^T window 1 + wvf early: lets vproj(4..7) run eagerly
            # while the 2MB wq/wk transfers are still in flight.
            nc.sync.dma_start(wvf_sb[:], wvf_d[:, :, :, :])
            nc.sync.dma_start(xf[:, :, QW:2 * QW], xf_d[:, :, QW:2 * QW])
            nc.scalar.dma_start(wq_sb[:], wq_d.rearrange("(o p) n -> p o n", p=P))
            nc.sync.dma_start(wk_sb[:], wk_d.rearrange("(o p) n -> p o n", p=P))
            nc.scalar.dma_start(bq_sb[:], bq_d.rearrange("(m p) -> p m", p=P))
            nc.sync.dma_start(bk_sb[:], bk_d.rearrange("(m p) -> p m", p=P))
            nc.scalar.dma_start(wqf_sb[:], wqf_d[:, :, :, :])
            nc.sync.dma_start(wkf_sb[:], wkf_d[:, :, :, :])
            for w in range(2, NQC):
                nc.scalar.dma_start(
                    xf[:, :, w * QW:(w + 1) * QW], xf_d[:, :, w * QW:(w + 1) * QW]
                )
            nc.scalar.dma_start(wo_sb[:], wo_d.rearrange("(f p) n -> p f n", p=P))
            nc.sync.dma_start(wof_sb[:], wof_d[:, :, :, :])
            # One-time scaled-bias copies for the fp8 q/k path.
            nc.vector.tensor_scalar_mul(bq32_sb[:], bq_sb[:], WSC)
            nc.vector.tensor_scalar_mul(bk32_sb[:], bk_sb[:], WSC)

            # V's softmax-sum column is constant 1.0 (weight 0, bias 1).
            for t in range(NT):
                nc.vector.memset(v_sb[:, t, :, HD], 1.0)

            # ---- PE clock warm-up ---------------------------------------
            # HAM un-throttles the PE (1.2 -> 2.4 GHz) after ~3.4us of
            # sustained matmul activity and re-throttles after ~3.4us idle.
            # Burn a burst of throwaway matmuls on the already-resident ut1
            # tile as soon as the engines come up, then keep the activity
            # window alive with drips gated on the first DMA arrivals, so
            # the first real projections run at full clock. Results are
            # never read; the m-pool rotation retires them naturally.
            for i in range(8):
                wps = m_pool.tile([P, P], F32, tag="m", name="warm")
                nc.tensor.matmul(wps[:], ut1[:], ut1[:], start=True, stop=True)
            for lhsT, rhs in (
                (xT[:, 0, 0:P], xT[:, 0, 0:256]),
                (xT[:, 0, 256:256 + P], xT[:, 0, 256:QW]),
                (wv_sb[:, 0, 0:P], wv_sb[:, 0, 0:256]),
            ):
                wps = m_pool.tile([P, 256], F32, tag="m", name="warmd")
                nc.tensor.matmul(wps[:], lhsT, rhs, start=True, stop=True)

            # ---- emitters ------------------------------------------------
            def vproj(t):
                """v for token tile t. Tiles >= 4 (tokens >= 512, which are
                only attended alongside >= 512 other keys) run fp8."""
                ps = m_pool.tile([P, C], F32, tag="m", name="psv")
                if t < 4:
                    for d in range(ND):
                        nc.tensor.matmul(
                            ps[:], xT[:, d, t * P:(t + 1) * P], wv_sb[:, d, :],
                            start=(d == 0), stop=(d == ND - 1),
                        )
                    nc.vector.tensor_add(v_sb[:, t, :, 0:HD], ps[:], bv_sb[:])
                else:
                    for u in range(ND // 2):
                        nc.tensor.matmul(
                            ps[:], xf[:, 2 * u:2 * u + 2, t * P:(t + 1) * P],
                            wvf_sb[:, u, :, :],
                            start=(u == 0), stop=(u == ND // 2 - 1),
                            perf_mode=mybir.MatmulPerfMode.DoubleRow,
                        )
                    nc.vector.scalar_tensor_tensor(
                        v_sb[:, t, :, 0:HD], ps[:], 1.0 / WSC, bv_sb[:],
                        mybir.AluOpType.mult, mybir.AluOpType.add,
                    )

            def qk_chunk(pair, win, which):
                """Projection of one 512-col window for q or k (both heads).

                Window 0 runs bf16; windows >= 1 run fp8 DoubleRow (two
                128-row contraction tiles per pass), leaving qTp/kTp scaled
                by WSC there — the per-tile EXP scale compensates."""
                sl = slice(win * QW, (win + 1) * QW)
                ps = m_pool.tile([P, QW], F32, tag="m", name="psqk")
                if win == 0:
                    wsb, bsb, dst = (
                        (wq_sb, bq_sb, qTp) if which == 0 else (wk_sb, bk_sb, kTp)
                    )
                    for d in range(ND):
                        nc.tensor.matmul(
                            ps[:], wsb[:, d, pair * P:(pair + 1) * P], xT[:, d, sl],
                            start=(d == 0), stop=(d == ND - 1),
                        )
                else:
                    wsb, bsb, dst = (
                        (wqf_sb, bq32_sb, qTp) if which == 0
                        else (wkf_sb, bk32_sb, kTp)
                    )
                    for t in range(ND // 2):
                        nc.tensor.matmul(
                            ps[:], wsb[:, t, :, pair * P:(pair + 1) * P],
                            xf[:, 2 * t:2 * t + 2, sl],
                            start=(t == 0), stop=(t == ND // 2 - 1),
                            perf_mode=mybir.MatmulPerfMode.DoubleRow,
                        )
                nc.vector.tensor_scalar_add(
                    dst[pair][:, sl], ps[:], bsb[:, pair:pair + 1])

            def oproj_chunk(qc, st, n2):
                m = qc * (QW // P) + st
                pso = m_pool.tile([P, 512], F32, tag="m", name="pso")
                ost = ost_pool.tile([P, 512], MD, tag="ost", name="ost")
                if qc == 0:
                    for pair in range(NP):
                        nc.tensor.matmul(
                            pso[:], stg[:, qc, pair, st * P:(st + 1) * P],
                            wo_sb[:, pair, n2 * 512:(n2 + 1) * 512],
                            start=(pair == 0), stop=(pair == NP - 1),
                        )
                    nc.vector.tensor_copy(ost[:], pso[:])
                else:
                    for t in range(NP // 2):
                        nc.tensor.matmul(
                            pso[:], stgf[:, qc, 2 * t:2 * t + 2, st * P:(st + 1) * P],
                            wof_sb[:, t, :, n2 * 512:(n2 + 1) * 512],
                            start=(t == 0), stop=(t == NP // 2 - 1),
                            perf_mode=mybir.MatmulPerfMode.DoubleRow,
                        )
                    nc.vector.tensor_scalar_mul(
                        ost[:], pso[:], 1.0 / (SSC * WSC))
                nc.sync.dma_start(
                    out_d[m * P:(m + 1) * P, n2 * 512:(n2 + 1) * 512], ost[:]
                )

            # ---- filler queue (debt-carrying pump) -----------------------
            fill = deque()
            done_keys = set()
            debt = [0.0]
            reserve = [0]

            def _pop_one():
                est, key, f = fill.popleft()
                f()
                if key is not None:
                    done_keys.add(key)
                debt[0] -= est

            def pump(ns):
                debt[0] = max(debt[0], -4000.0) + ns
                while debt[0] > 0 and len(fill) > reserve[0]:
                    _pop_one()

            def pump_until(key):
                while fill and key not in done_keys:
                    _pop_one()

            def seed_qk(pair, win):
                est = 2100
                for which in (0, 1):
                    key = ("qk", pair, win) if which == 1 else None
                    fill.append((
                        est, key,
                        lambda pair=pair, win=win, which=which:
                            qk_chunk(pair, win, which),
                    ))

            def seed_v(t0, t1):
                for t in range(t0, t1):
                    fill.append((2100, ("v", t), lambda t=t: vproj(t)))

            def seed_oproj(qc):
                est = 1100
                for st in range(QW // P):
                    for n2 in range(2):
                        fill.append((
                            est, None,
                            lambda qc=qc, st=st, n2=n2: oproj_chunk(qc, st, n2),
                        ))

            # ---- attention (one head pair, one query window) -------------
            def att(pair, qc, pre=None):
                W0 = qc * QW
                NK = W0 // P + 4
                o01 = o_pool.tile([VW, 2, QW], F32, tag="o", name="o01")
                pending = None

                def emit_pv(ki, pT_t, rel):
                    if ("v", ki) in v_guard:
                        pump_until(("v", ki))
                    # For diagonal tiles, emit the mask-free columns first so
                    # only a 128-col piece waits on the DVE mask multiply.
                    # Never split the ki==0 matmul: its start=True clears the
                    # whole PSUM bank, so a second start=True piece in the
                    # same bank would wipe the first piece's output.
                    pieces = [(rel, QW)]
                    if ki > 0 and ki * P >= W0 and rel + P < QW:
                        pieces = [(rel + P, QW), (rel, rel + P)]
                    for hh in (0, 1):
                        for lo, hi in pieces:
                            nc.tensor.matmul(
                                o01[:, hh, lo:hi], v_sb[:, ki, 2 * pair + hh, :],
                                pT_t[:, hh, lo:hi],
                                start=(ki == 0), stop=(ki == NK - 1),
                            )

                for ki in range(NK):
                    rel = max(W0, ki * P) - W0
                    w = QW - rel
                    ko = ki * P
                    s_ps = s_pool.tile([P, 2, QW], F32, tag="s", name="s_ps")
                    qs = slice(W0 + rel, W0 + QW)
                    nc.tensor.matmul(
                        s_ps[0:64, 0, rel:QW], kTp[pair][0:64, ko:ko + 64],
                        qTp[pair][0:64, qs], start=True, stop=True,
                        tile_position=(0, 0),
                    )
                    nc.tensor.matmul(
                        s_ps[64:128, 0, rel:QW], kTp[pair][0:64, ko + 64:ko + 128],
                        qTp[pair][0:64, qs], start=True, stop=True,
                        tile_position=(0, 64),
                    )
                    nc.tensor.matmul(
                        s_ps[0:64, 1, rel:QW], kTp[pair][64:128, ko:ko + 64],
                        qTp[pair][64:128, qs], start=True, stop=True,
                        tile_position=(64, 0),
                    )
                    nc.tensor.matmul(
                        s_ps[64:128, 1, rel:QW], kTp[pair][64:128, ko + 64:ko + 128],
                        qTp[pair][64:128, qs], start=True, stop=True,
                        tile_position=(64, 64),
                    )
                    pT_t = pT_pool.tile([P, 2, QW], MD, tag="pT", name="pT")
                    # Divide out the fp8 weight pre-scale for whichever of
                    # the q/k windows feeding this tile ran the fp8 path.
                    sdiv = (WSC if qc >= 1 else 1.0) * (
                        WSC if ki >= QW // P else 1.0)
                    nc.scalar.activation(
                        pT_t[:, :, rel:QW], s_ps[:, :, rel:QW], AF.Exp,
                        scale=SCALE / sdiv,
                    )
                    if ko >= W0:
                        # One DVE multiply masks both heads: broadcast ut1
                        # across the head plane with a 0-stride free dim.
                        u1 = ut1[:]
                        u1b = bass.AP(
                            tensor=u1.tensor, offset=u1.offset,
                            ap=[u1.ap[0], [0, 2], u1.ap[1]],
                        )
                        nc.vector.tensor_mul(
                            pT_t[:, :, rel:rel + P], pT_t[:, :, rel:rel + P], u1b)
                    if pending is not None:
                        emit_pv(*pending)
                    if ki == 2 and pre is not None:
                        pre()
                    # deficit pump: cover EXP latency minus this iteration's
                    # own PE work (quad score group measured ~232ns at w=512,
                    # PV ~250ns/head incl. weight loads => ~1.43*w total).
                    # Debt carry-over in pump() smooths over-/under-shoot.
                    exp_ns = (2 * w + 352) / 1.2
                    pe_ns = 1.55 * w
                    pump(int(max(0.0, exp_ns - pe_ns)))
                    pending = (ki, pT_t, rel)
                emit_pv(*pending)

                # normalization: copy both heads' O^T+sums to SBUF, 1/sums
                # via the single-op DVE approx reciprocal (~18 bits, off the
                # EXP-critical ACT engine), broadcast across the 64 head-dim
                # partitions on the otherwise-idle GpSimd, then a deferred
                # DVE multiply into staging (SSC fold rides the multiply).
                ocp = ocp_pool.tile([VW, 2, QW], F32, tag="ocp", name="ocp")
                nc.vector.tensor_copy(ocp[:], o01[:])
                # Stage the sums rows onto partition 0 via ACT straight from
                # PSUM (parallel to the DVE copies above): the custom DVE
                # reciprocal ignores partition offsets, so it must read a
                # partition-0 source.
                sums0 = rc_pool.tile([1, 2, QW], F32, tag="lg", name="sums0")
                nc.scalar.copy(sums0[:], o01[HD:HD + 1, :, :])
                rcx = rc_pool.tile([1, 2, QW], F32, tag="rc", name="rc")
                nc.vector.reciprocal_approx_fast(rcx[:], sums0[:])
                bc = bc_pool.tile([HD, 2, QW], F32, tag="bc", name="bc")
                nc.gpsimd.partition_broadcast(bc[:], rcx[:])
                pump(2000)

                def finish_norm():
                    dst = stg if qc == 0 else stgf
                    ssc = SSC if qc >= 1 else 1.0
                    for hh in (0, 1):
                        nc.vector.scalar_tensor_tensor(
                            dst[hh * HD:(hh + 1) * HD, qc, pair, :],
                            ocp[0:HD, hh, :], ssc, bc[:, hh, :],
                            mybir.AluOpType.mult, mybir.AluOpType.mult,
                        )
                return finish_norm

            # ---- main schedule -------------------------------------------
            v_guard = set()
            # v tiles 0-7 run eagerly: 0-3 (bf16) gate only wv + x^T window
            # 0, and 4-7 (fp8) cover the PE while the 2MB wq/wk DMAs land.
            for t in range(8):
                vproj(t)
            for pair in range(NP):
                qk_chunk(pair, 0, 0)
                qk_chunk(pair, 0, 1)
                done_keys.add(("qk", pair, 0))

            pending_norm = None
            for qc in range(NQC):
                # seed supply consumed during this window's attention
                if qc + 1 < NQC:
                    for pair in range(NP):
                        seed_qk(pair, qc + 1)
                    t0, t1 = 4 * (qc + 2), min(4 * (qc + 3), NT)
                    if t0 < NT:
                        seed_v(t0, t1)
                        v_guard.update(("v", t) for t in range(t0, t1))
                # Hold back filler in the middle windows (which have surplus
                # supply) so the late windows — whose just-in-time supply is
                # thin — don't starve the PE between EXPs.
                if qc in (1, 2):
                    reserve[0] = 8
                if qc == NQC - 1:
                    reserve[0] = 4
                for pair in range(NP):
                    pump_until(("qk", pair, qc))
                    # Prefetch the next pair's projections too: their PSUM
                    # drain then completes during this pair's attention, so
                    # the next pair's first score matmul starts stall-free.
                    if pair + 1 < NP:
                        pump_until(("qk", pair + 1, qc))
                    pending_norm = att(pair, qc, pre=pending_norm)
                reserve[0] = 0
                pump(3000)
                pending_norm()
                pending_norm = None
                seed_oproj(qc)
            pump(10**9)

    nc.compile()
    return nc


_NC_CACHE = {}


def _get_nc(S, D, HN, HD):
    key = (S, D, HN, HD)
    if key not in _NC_CACHE:
        _NC_CACHE[key] = build_nc(S, D, HN, HD)
    return _NC_CACHE[key]


def kernel(**inputs):
    out, _ = run_with_results(inputs)
    return out


def run_with_results(inputs, **spmd_kwargs):
    from concourse.bass_utils import run_bass_kernel_spmd

    x = np.asarray(inputs["x"], dtype=np.float32)
    wq = np.asarray(inputs["wq"], dtype=np.float32)
    bq = np.asarray(inputs["bq"], dtype=np.float32)
    wk = np.asarray(inputs["wk"], dtype=np.float32)
    bk = np.asarray(inputs["bk"], dtype=np.float32)
    wv = np.asarray(inputs["wv"], dtype=np.float32)
    bv = np.asarray(inputs["bv"], dtype=np.float32)
    wo = np.asarray(inputs["wo"], dtype=np.float32)
    bo = np.asarray(inputs["bo"], dtype=np.float32)

    B, S, D = x.shape
    H = 16
    HD = D // H
    G = 2
    HN = H // G
    C = HN * HD
    n_cores = B * G

    nc = _get_nc(S, D, HN, HD)
    np_md = mybir.dt.np(BF16)
    np_f8 = mybir.dt.np(FP8)
    PP = 128

    def pack_w_f8(w):
        # [K, N] -> [128, K//256, 2, N] with wf[p, t, j, n] = WSC*w[256t+128j+p, n]
        return np.ascontiguousarray(
            (w * WSC).reshape(w.shape[0] // 256, 2, PP, w.shape[1])
            .transpose(2, 0, 1, 3)
        ).astype(np_f8)

    in_maps = []
    for c in range(n_cores):
        b, g = c // G, c % G
        sl = slice(g * C, (g + 1) * C)
        xTb = np.ascontiguousarray(x[b].T)
        in_maps.append({
            "xT": xTb.astype(np_md),
            "xf": np.ascontiguousarray(
                xTb.reshape(D // PP, PP, S).transpose(1, 0, 2)).astype(np_f8),
            "wq": np.ascontiguousarray(wq[:, sl]).astype(np_md),
            "wk": np.ascontiguousarray(wk[:, sl]).astype(np_md),
            "wqf": pack_w_f8(wq[:, sl]),
            "wkf": pack_w_f8(wk[:, sl]),
            "wvf": pack_w_f8(wv[:, sl]),
            "wof": pack_w_f8(wo[sl, :]),
            "wv": np.ascontiguousarray(wv[:, sl]).astype(np_md),
            "wo": np.ascontiguousarray(wo[sl, :]).astype(np_md),
            "bq": np.ascontiguousarray(bq[sl]),
            "bk": np.ascontiguousarray(bk[sl]),
            "bv": np.ascontiguousarray(bv[sl]),
        })

    res = run_bass_kernel_spmd(nc, in_maps, core_ids=list(range(n_cores)), **spmd_kwargs)
    outs = [np.asarray(m["out"]).astype(np.float32) for m in res.results]
    out = np.stack([sum(outs[b * G + g] for g in range(G)) for b in range(B)])
    out = out + bo[None, None, :]
    return out.astype(np.float32), res



# revision 84
# speedup vs baseline: 1.0149x; 1.0149x over previous
"""Trainium2 Bass kernel for nn_MultiHeadAttention (B=4, S=2048, D=1024, H=16).

Sharding: 8 cores = 4 batches x 2 head-groups. Host ships x^T (bf16) so the
kernel never transposes on-chip. Each core: Q/K/V projections for its 512
columns, causal attention for its 8 heads, partial output projection
(row-parallel over wo); host sums the two partials per batch and adds bo.

v4 additions over v3:
  - fp8 (e4m3, DoubleRow 2-weights/cell) projections for everything that
    only feeds long-context queries: q/k/out-proj for query windows 1-3 and
    v for token tiles >= 4. Those paths' fp8 noise is suppressed ~1/sqrt(k)
    by softmax averaging over >= 512 keys; window 0 (short-context queries,
    no suppression) stays bf16 end to end. Weights pre-scaled x32 (and the
    fp8 out-proj staging x16) to sit in e4m3's normal range, divided back
    out in the EXP scale / PSUM drains. Cuts ~17us of PE streaming.
  - Output ships bf16 (host upcasts + sums partials in fp32).
  - Startup DMA order puts wv + x^T window 0 first; v tiles 0-7 run eagerly
    (4-7 via fp8) to cover the wq/wk DMA shadow; mid-window filler reserve
    (8 items at qc=1,2) guards the late windows against starvation; each
    pair's projections are force-pumped one attention-call ahead so their
    PSUM drain completes before the scores need them.
  - Softmax epilogue off the ACT/PE critical paths: ACT stages the sums
    rows to partition 0 (custom DVE ops ignore partition offsets), DVE
    reciprocal_approx_fast computes 1/sums, the otherwise-idle GpSimd
    broadcasts it across the head-dim partitions (partition_broadcast
    ucode), and one fused DVE scalar_tensor_tensor normalizes + applies the
    fp8 staging scale. Replaces the ACT ln/exp pair (-36us ACT) and the PE
    K=1 broadcast matmuls (-11us PE) of v3.

v3 design (all bf16 through the PE, fp32 PSUM):
  - Scores quadrant-packed: both heads of a pair computed concurrently in the
    128x128 PE array via 2x2 tile_position tiling with [64,64] stationaries
    (NumWeights=64 keeps FWL off - the bf16 base-64 128-col FWL path crashes).
    Halves score-matmul wall time.
  - q-window-outer (512 queries), pair-inner attention; one EXP instruction
    covers both heads' scores ([128, 2, w]).
  - Softmax 1/sums = exp(-ln(sums)) on ACT (ln+exp pinned to the
    natural_log_exp_and_others table set - avoids 32 table reloads);
    PE K=1 ones-matmul broadcasts across partitions; DVE multiply writes
    the bf16 staging tile. V's sums column is a memset constant.
  - Deficit-based filler pump: each attention iteration emits just enough
    projection/output-projection matmuls to cover the ACT exp latency, so
    the PE stream stays dense and the HAM clock gate stays at K=8/8.
"""

from collections import deque

import numpy as np

import concourse.bass as bass
import concourse.mybir as mybir
import concourse.tile as tile
from concourse import bacc
from concourse.masks import make_upper_triangular

F32 = mybir.dt.float32
BF16 = mybir.dt.bfloat16
FP8 = mybir.dt.float8e4
P = 128
AF = mybir.ActivationFunctionType
# q/k/v/output projections for token windows >= 1 run in fp8 (DoubleRow,
# 2x PE rate). Weights are pre-scaled by WSC so N(0, 1/32) values sit in
# e4m3's normal range; the factor is divided back out downstream (EXP scale
# for scores, the PSUM drain for v / out-proj). The attention-output staging
# for fp8 out-proj is pre-scaled by SSC so its ~N(0, 1/k_eff) values stay
# normal too. Window 0 (queries 0-511, which attend few keys and get no
# averaging suppression of the fp8 noise) stays bf16 end to end.
WSC = 32.0
SSC = 16.0


def _pin_act_tables(arch):
    """Mutate the cached activation-table map so Exp and Ln both resolve to
    natural_log_exp_and_others (set indices preserved -> one table load)."""
    try:
        from concourse.hw_specs import get_activation_tables
        tabs = get_activation_tables(arch)
    except Exception:
        return
    pin = tabs.get("natural_log_exp_and_others")
    if not pin or AF.Exp not in pin or AF.Ln not in pin:
        return
    for name, funcs in tabs.items():
        if name != "natural_log_exp_and_others":
            funcs.discard(AF.Exp)
            funcs.discard(AF.Ln)


def build_nc(S=2048, D=1024, HN=8, HD=64):
    MD = BF16
    C = HN * HD        # 512 local head-dims
    NT = S // P        # 16 token tiles
    ND = D // P        # 8 contraction tiles for projections
    NP = HN // 2       # 4 head pairs
    QW = 512           # query window (1 PSUM bank per head)
    NQC = S // QW      # 4
    VW = HD + 1
    SCALE = 1.0 / float(np.sqrt(HD))

    nc = bacc.Bacc("TRN2", target_bir_lowering=False)
    _pin_act_tables(nc.m.arch)

    xT_d = nc.dram_tensor("xT", [D, S], MD, kind="ExternalInput")
    wq_d = nc.dram_tensor("wq", [D, C], MD, kind="ExternalInput")
    wk_d = nc.dram_tensor("wk", [D, C], MD, kind="ExternalInput")
    # fp8 copies, host-packed for DoubleRow: x^T pre-arranged [P, ND, S];
    # weights [P, ND/2, 2, C] with wf[p, t, j, c] = WSC*w[256t + 128j + p, c].
    xf_d = nc.dram_tensor("xf", [P, D // P, S], FP8, kind="ExternalInput")
    wqf_d = nc.dram_tensor("wqf", [P, D // 256, 2, C], FP8, kind="ExternalInput")
    wkf_d = nc.dram_tensor("wkf", [P, D // 256, 2, C], FP8, kind="ExternalInput")
    wvf_d = nc.dram_tensor("wvf", [P, D // 256, 2, C], FP8, kind="ExternalInput")
    wof_d = nc.dram_tensor("wof", [P, C // 256, 2, D], FP8, kind="ExternalInput")
    wv_d = nc.dram_tensor("wv", [D, C], MD, kind="ExternalInput")
    wo_d = nc.dram_tensor("wo", [C, D], MD, kind="ExternalInput")
    bq_d = nc.dram_tensor("bq", [C], F32, kind="ExternalInput")
    bk_d = nc.dram_tensor("bk", [C], F32, kind="ExternalInput")
    bv_d = nc.dram_tensor("bv", [C], F32, kind="ExternalInput")
    # Output ships as bf16 (host upcasts and sums the two per-batch
    # partials in fp32): halves output DMA traffic; adds <=0.4% rounding
    # on top of a ~0.4% bf16 pipeline, well inside the error budget.
    out_d = nc.dram_tensor("out", [S, D], MD, kind="ExternalOutput")

    with tile.TileContext(nc) as tc:
        from contextlib import ExitStack

        with ExitStack() as ctx:
            singles = ctx.enter_context(tc.tile_pool(name="singles", bufs=1))
            # ut1[k, q] = 1.0 where k <= q (valid causal region of a diagonal
            # tile in S^T = [k, q] layout).
            ut1 = singles.tile([P, P], F32)
            make_upper_triangular(nc, ut1[:], val=1.0, diag=True)
            ones1 = singles.tile([1, HD], MD)
            nc.vector.memset(ones1[:], 1.0)
            bq_sb = singles.tile([P, NP], F32)
            bk_sb = singles.tile([P, NP], F32)
            bv_sb = singles.tile([P, C], F32)
            bq32_sb = singles.tile([P, NP], F32)
            bk32_sb = singles.tile([P, NP], F32)

            wq_pool = ctx.enter_context(tc.tile_pool(name="wq", bufs=1))
            wq_sb = wq_pool.tile([P, ND, C], MD)
            wk_sb = wq_pool.tile([P, ND, C], MD)
            wv_sb = wq_pool.tile([P, ND, C], MD)
            wo_sb = wq_pool.tile([P, NP, D], MD)
            wqf_sb = wq_pool.tile([P, ND // 2, 2, C], FP8)
            wkf_sb = wq_pool.tile([P, ND // 2, 2, C], FP8)
            wvf_sb = wq_pool.tile([P, ND // 2, 2, C], FP8)
            wof_sb = wq_pool.tile([P, NP // 2, 2, D], FP8)

            xT_pool = ctx.enter_context(tc.tile_pool(name="xT", bufs=1))
            # bf16 x^T only covers window 0 (the bf16 projection paths);
            # windows 1-3 are consumed exclusively through the fp8 copy.
            xT = xT_pool.tile([P, ND, QW], MD)
            xf = xT_pool.tile([P, ND, S], FP8)

            v_pool = ctx.enter_context(tc.tile_pool(name="v", bufs=1))
            v_sb = v_pool.tile([P, NT, HN, VW], MD)

            qkT_pool = ctx.enter_context(tc.tile_pool(name="qkT", bufs=1))
            qTp = [qkT_pool.tile([P, S], MD, name=f"qTp{p}") for p in range(NP)]
            kTp = [qkT_pool.tile([P, S], MD, name=f"kTp{p}") for p in range(NP)]

            stg_pool = ctx.enter_context(tc.tile_pool(name="stg", bufs=1))
            stg = stg_pool.tile([P, NQC, NP, QW], MD)
            stgf = stg_pool.tile([P, NQC, NP, QW], FP8)

            pT_pool = ctx.enter_context(tc.tile_pool(name="pT", bufs=4))
            ocp_pool = ctx.enter_context(tc.tile_pool(name="ocp", bufs=2))
            rc_pool = ctx.enter_context(tc.tile_pool(name="rc", bufs=2))
            bc_pool = ctx.enter_context(tc.tile_pool(name="bc", bufs=2))
            ost_pool = ctx.enter_context(tc.tile_pool(name="ost", bufs=4))

            # PSUM: scores 2 slots x [P,2,512] (2 banks each) + O^T 2 x 1 bank
            # + misc (projections / broadcast / out-proj) 2 x 1 bank = 8.
            s_pool = ctx.enter_context(tc.tile_pool(name="ps_s", bufs=2, space="PSUM"))
            o_pool = ctx.enter_context(tc.tile_pool(name="ps_o", bufs=1, space="PSUM"))
            m_pool = ctx.enter_context(tc.tile_pool(name="ps_m", bufs=2, space="PSUM"))

            # ---- DMA schedule (priority order) ---------------------------
            # Input DMAs split across the two HWDGE rings (sync + scalar) so
            # wv and the first x^T window transfer in parallel at startup.
            # Biases follow the first compute's inputs: they are only needed
            # by the PSUM drains, ~2us after the first matmul.
            xr = xT_d.rearrange("(o p) n -> p o n", p=P)
            wvr = wv_d.rearrange("(o p) n -> p o n", p=P)
            nc.scalar.dma_start(xT[:, :, 0:256], xr[:, :, 0:256])
            nc.sync.dma_start(wv_sb[:, 0:4, :], wvr[:, 0:4, :])
            nc.scalar.dma_start(xT[:, :, 256:QW], xr[:, :, 256:QW])
            nc.sync.dma_start(wv_sb[:, 4:8, :], wvr[:, 4:8, :])
            nc.sync.dma_start(
                bv_sb[:], bass.AP(tensor=bv_d, offset=0, ap=[[0, P], [1, C]])
            )
            # fp8 x^T window 1 + wvf early: lets vproj(4..7) run eagerly
            # while the 2MB wq/wk transfers are still in flight.
            nc.sync.dma_start(wvf_sb[:], wvf_d[:, :, :, :])
            nc.sync.dma_start(xf[:, :, QW:2 * QW], xf_d[:, :, QW:2 * QW])
            nc.scalar.dma_start(wq_sb[:], wq_d.rearrange("(o p) n -> p o n", p=P))
            nc.sync.dma_start(wk_sb[:], wk_d.rearrange("(o p) n -> p o n", p=P))
            nc.scalar.dma_start(bq_sb[:], bq_d.rearrange("(m p) -> p m", p=P))
            nc.sync.dma_start(bk_sb[:], bk_d.rearrange("(m p) -> p m", p=P))
            nc.scalar.dma_start(wqf_sb[:], wqf_d[:, :, :, :])
            nc.sync.dma_start(wkf_sb[:], wkf_d[:, :, :, :])
            for w in range(2, NQC):
                nc.scalar.dma_start(
                    xf[:, :, w * QW:(w + 1) * QW], xf_d[:, :, w * QW:(w + 1) * QW]
                )
            nc.scalar.dma_start(wo_sb[:], wo_d.rearrange("(f p) n -> p f n", p=P))
            nc.sync.dma_start(wof_sb[:], wof_d[:, :, :, :])
            # One-time scaled-bias copies for the fp8 q/k path.
            nc.vector.tensor_scalar_mul(bq32_sb[:], bq_sb[:], WSC)
            nc.vector.tensor_scalar_mul(bk32_sb[:], bk_sb[:], WSC)

            # V's softmax-sum column is constant 1.0 (weight 0, bias 1).
            for t in range(NT):
                nc.vector.memset(v_sb[:, t, :, HD], 1.0)

            # ---- emitters ------------------------------------------------
            def vproj(t):
                """v for token tile t. Tiles >= 4 (tokens >= 512, which are
                only attended alongside >= 512 other keys) run fp8."""
                ps = m_pool.tile([P, C], F32, tag="m", name="psv")
                if t < 4:
                    for d in range(ND):
                        nc.tensor.matmul(
                            ps[:], xT[:, d, t * P:(t + 1) * P], wv_sb[:, d, :],
                            start=(d == 0), stop=(d == ND - 1),
                        )
                    nc.vector.tensor_add(v_sb[:, t, :, 0:HD], ps[:], bv_sb[:])
                else:
                    for u in range(ND // 2):
                        nc.tensor.matmul(
                            ps[:], xf[:, 2 * u:2 * u + 2, t * P:(t + 1) * P],
                            wvf_sb[:, u, :, :],
                            start=(u == 0), stop=(u == ND // 2 - 1),
                            perf_mode=mybir.MatmulPerfMode.DoubleRow,
                        )
                    nc.vector.scalar_tensor_tensor(
                        v_sb[:, t, :, 0:HD], ps[:], 1.0 / WSC, bv_sb[:],
                        mybir.AluOpType.mult, mybir.AluOpType.add,
                    )

            def qk_chunk(pair, win, which):
                """Projection of one 512-col window for q or k (both heads).

                Window 0 runs bf16; windows >= 1 run fp8 DoubleRow (two
                128-row contraction tiles per pass), leaving qTp/kTp scaled
                by WSC there — the per-tile EXP scale compensates."""
                sl = slice(win * QW, (win + 1) * QW)
                ps = m_pool.tile([P, QW], F32, tag="m", name="psqk")
                if win == 0:
                    wsb, bsb, dst = (
                        (wq_sb, bq_sb, qTp) if which == 0 else (wk_sb, bk_sb, kTp)
                    )
                    for d in range(ND):
                        nc.tensor.matmul(
                            ps[:], wsb[:, d, pair * P:(pair + 1) * P], xT[:, d, sl],
                            start=(d == 0), stop=(d == ND - 1),
                        )
                else:
                    wsb, bsb, dst = (
                        (wqf_sb, bq32_sb, qTp) if which == 0
                        else (wkf_sb, bk32_sb, kTp)
                    )
                    for t in range(ND // 2):
                        nc.tensor.matmul(
                            ps[:], wsb[:, t, :, pair * P:(pair + 1) * P],
                            xf[:, 2 * t:2 * t + 2, sl],
                            start=(t == 0), stop=(t == ND // 2 - 1),
                            perf_mode=mybir.MatmulPerfMode.DoubleRow,
                        )
                nc.vector.tensor_scalar_add(
                    dst[pair][:, sl], ps[:], bsb[:, pair:pair + 1])

            def oproj_chunk(qc, st, n2):
                m = qc * (QW // P) + st
                pso = m_pool.tile([P, 512], F32, tag="m", name="pso")
                ost = ost_pool.tile([P, 512], MD, tag="ost", name="ost")
                if qc == 0:
                    for pair in range(NP):
                        nc.tensor.matmul(
                            pso[:], stg[:, qc, pair, st * P:(st + 1) * P],
                            wo_sb[:, pair, n2 * 512:(n2 + 1) * 512],
                            start=(pair == 0), stop=(pair == NP - 1),
                        )
                    nc.vector.tensor_copy(ost[:], pso[:])
                else:
                    for t in range(NP // 2):
                        nc.tensor.matmul(
                            pso[:], stgf[:, qc, 2 * t:2 * t + 2, st * P:(st + 1) * P],
                            wof_sb[:, t, :, n2 * 512:(n2 + 1) * 512],
                            start=(t == 0), stop=(t == NP // 2 - 1),
                            perf_mode=mybir.MatmulPerfMode.DoubleRow,
                        )
                    nc.vector.tensor_scalar_mul(
                        ost[:], pso[:], 1.0 / (SSC * WSC))
                nc.sync.dma_start(
                    out_d[m * P:(m + 1) * P, n2 * 512:(n2 + 1) * 512], ost[:]
                )

            # ---- filler queue (debt-carrying pump) -----------------------
            fill = deque()
            done_keys = set()
            debt = [0.0]
            reserve = [0]

            def _pop_one():
                est, key, f = fill.popleft()
                f()
                if key is not None:
                    done_keys.add(key)
                debt[0] -= est

            def pump(ns):
                debt[0] = max(debt[0], -4000.0) + ns
                while debt[0] > 0 and len(fill) > reserve[0]:
                    _pop_one()

            def pump_until(key):
                while fill and key not in done_keys:
                    _pop_one()

            def seed_qk(pair, win):
                est = 2100
                for which in (0, 1):
                    key = ("qk", pair, win) if which == 1 else None
                    fill.append((
                        est, key,
                        lambda pair=pair, win=win, which=which:
                            qk_chunk(pair, win, which),
                    ))

            def seed_v(t0, t1):
                for t in range(t0, t1):
                    fill.append((2100, ("v", t), lambda t=t: vproj(t)))

            def seed_oproj(qc):
                est = 1100
                for st in range(QW // P):
                    for n2 in range(2):
                        fill.append((
                            est, None,
                            lambda qc=qc, st=st, n2=n2: oproj_chunk(qc, st, n2),
                        ))

            # ---- attention (one head pair, one query window) -------------
            def att(pair, qc, pre=None):
                W0 = qc * QW
                NK = W0 // P + 4
                o01 = o_pool.tile([VW, 2, QW], F32, tag="o", name="o01")
                pending = None

                def emit_pv(ki, pT_t, rel):
                    if ("v", ki) in v_guard:
                        pump_until(("v", ki))
                    # For diagonal tiles, emit the mask-free columns first so
                    # only a 128-col piece waits on the DVE mask multiply.
                    # Never split the ki==0 matmul: its start=True clears the
                    # whole PSUM bank, so a second start=True piece in the
                    # same bank would wipe the first piece's output.
                    pieces = [(rel, QW)]
                    if ki > 0 and ki * P >= W0 and rel + P < QW:
                        pieces = [(rel + P, QW), (rel, rel + P)]
                    for hh in (0, 1):
                        for lo, hi in pieces:
                            nc.tensor.matmul(
                                o01[:, hh, lo:hi], v_sb[:, ki, 2 * pair + hh, :],
                                pT_t[:, hh, lo:hi],
                                start=(ki == 0), stop=(ki == NK - 1),
                            )

                for ki in range(NK):
                    rel = max(W0, ki * P) - W0
                    w = QW - rel
                    ko = ki * P
                    s_ps = s_pool.tile([P, 2, QW], F32, tag="s", name="s_ps")
                    qs = slice(W0 + rel, W0 + QW)
                    nc.tensor.matmul(
                        s_ps[0:64, 0, rel:QW], kTp[pair][0:64, ko:ko + 64],
                        qTp[pair][0:64, qs], start=True, stop=True,
                        tile_position=(0, 0),
                    )
                    nc.tensor.matmul(
                        s_ps[64:128, 0, rel:QW], kTp[pair][0:64, ko + 64:ko + 128],
                        qTp[pair][0:64, qs], start=True, stop=True,
                        tile_position=(0, 64),
                    )
                    nc.tensor.matmul(
                        s_ps[0:64, 1, rel:QW], kTp[pair][64:128, ko:ko + 64],
                        qTp[pair][64:128, qs], start=True, stop=True,
                        tile_position=(64, 0),
                    )
                    nc.tensor.matmul(
                        s_ps[64:128, 1, rel:QW], kTp[pair][64:128, ko + 64:ko + 128],
                        qTp[pair][64:128, qs], start=True, stop=True,
                        tile_position=(64, 64),
                    )
                    pT_t = pT_pool.tile([P, 2, QW], MD, tag="pT", name="pT")
                    # Divide out the fp8 weight pre-scale for whichever of
                    # the q/k windows feeding this tile ran the fp8 path.
                    sdiv = (WSC if qc >= 1 else 1.0) * (
                        WSC if ki >= QW // P else 1.0)
                    nc.scalar.activation(
                        pT_t[:, :, rel:QW], s_ps[:, :, rel:QW], AF.Exp,
                        scale=SCALE / sdiv,
                    )
                    if ko >= W0:
                        # One DVE multiply masks both heads: broadcast ut1
                        # across the head plane with a 0-stride free dim.
                        u1 = ut1[:]
                        u1b = bass.AP(
                            tensor=u1.tensor, offset=u1.offset,
                            ap=[u1.ap[0], [0, 2], u1.ap[1]],
                        )
                        nc.vector.tensor_mul(
                            pT_t[:, :, rel:rel + P], pT_t[:, :, rel:rel + P], u1b)
                    if pending is not None:
                        emit_pv(*pending)
                    if ki == 2 and pre is not None:
                        pre()
                    # deficit pump: cover EXP latency minus this iteration's
                    # own PE work (quad score group measured ~232ns at w=512,
                    # PV ~250ns/head incl. weight loads => ~1.43*w total).
                    # Debt carry-over in pump() smooths over-/under-shoot.
                    exp_ns = (2 * w + 352) / 1.2
                    pe_ns = 1.55 * w
                    pump(int(max(0.0, exp_ns - pe_ns)))
                    pending = (ki, pT_t, rel)
                emit_pv(*pending)

                # normalization: copy both heads' O^T+sums to SBUF, 1/sums
                # via the single-op DVE approx reciprocal (~18 bits, off the
                # EXP-critical ACT engine), broadcast across the 64 head-dim
                # partitions on the otherwise-idle GpSimd, then a deferred
                # DVE multiply into staging (SSC fold rides the multiply).
                ocp = ocp_pool.tile([VW, 2, QW], F32, tag="ocp", name="ocp")
                nc.vector.tensor_copy(ocp[:], o01[:])
                # Stage the sums rows onto partition 0 via ACT straight from
                # PSUM (parallel to the DVE copies above): the custom DVE
                # reciprocal ignores partition offsets, so it must read a
                # partition-0 source.
                sums0 = rc_pool.tile([1, 2, QW], F32, tag="lg", name="sums0")
                nc.scalar.copy(sums0[:], o01[HD:HD + 1, :, :])
                rcx = rc_pool.tile([1, 2, QW], F32, tag="rc", name="rc")
                nc.vector.reciprocal_approx_fast(rcx[:], sums0[:])
                bc = bc_pool.tile([HD, 2, QW], F32, tag="bc", name="bc")
                nc.gpsimd.partition_broadcast(bc[:], rcx[:])
                pump(2000)

                def finish_norm():
                    dst = stg if qc == 0 else stgf
                    ssc = SSC if qc >= 1 else 1.0
                    for hh in (0, 1):
                        nc.vector.scalar_tensor_tensor(
                            dst[hh * HD:(hh + 1) * HD, qc, pair, :],
                            ocp[0:HD, hh, :], ssc, bc[:, hh, :],
                            mybir.AluOpType.mult, mybir.AluOpType.mult,
                        )
                return finish_norm

            # ---- main schedule -------------------------------------------
            v_guard = set()
            # v tiles 0-7 run eagerly: 0-3 (bf16) gate only wv + x^T window
            # 0, and 4-7 (fp8) cover the PE while the 2MB wq/wk DMAs land.
            for t in range(8):
                vproj(t)
            for pair in range(NP):
                qk_chunk(pair, 0, 0)
                qk_chunk(pair, 0, 1)
                done_keys.add(("qk", pair, 0))

            pending_norm = None
            for qc in range(NQC):
                # seed supply consumed during this window's attention
                if qc + 1 < NQC:
                    for pair in range(NP):
                        seed_qk(pair, qc + 1)
                    t0, t1 = 4 * (qc + 2), min(4 * (qc + 3), NT)
                    if t0 < NT:
                        seed_v(t0, t1)
                        v_guard.update(("v", t) for t in range(t0, t1))
                # Hold back filler in the middle windows (which have surplus
                # supply) so the late windows — whose just-in-time supply is
                # thin — don't starve the PE between EXPs.
                if qc in (1, 2):
                    reserve[0] = 8
                if qc == NQC - 1:
                    reserve[0] = 4
                for pair in range(NP):
                    pump_until(("qk", pair, qc))
                    # Prefetch the next pair's projections too: their PSUM
                    # drain then completes during this pair's attention, so
                    # the next pair's first score matmul starts stall-free.
                    if pair + 1 < NP:
                        pump_until(("qk", pair + 1, qc))
                    pending_norm = att(pair, qc, pre=pending_norm)
                reserve[0] = 0
                pump(3000)
                pending_norm()
                pending_norm = None
                seed_oproj(qc)
            pump(10**9)

    nc.compile()
    return nc


_NC_CACHE = {}


def _get_nc(S, D, HN, HD):
    key = (S, D, HN, HD)
    if key not in _NC_CACHE:
        _NC_CACHE[key] = build_nc(S, D, HN, HD)
    return _NC_CACHE[key]


def kernel(**inputs):
    out, _ = run_with_results(inputs)
    return out


def run_with_results(inputs, **spmd_kwargs):
    from concourse.bass_utils import run_bass_kernel_spmd

    x = np.asarray(inputs["x"], dtype=np.float32)
    wq = np.asarray(inputs["wq"], dtype=np.float32)
    bq = np.asarray(inputs["bq"], dtype=np.float32)
    wk = np.asarray(inputs["wk"], dtype=np.float32)
    bk = np.asarray(inputs["bk"], dtype=np.float32)
    wv = np.asarray(inputs["wv"], dtype=np.float32)
    bv = np.asarray(inputs["bv"], dtype=np.float32)
    wo = np.asarray(inputs["wo"], dtype=np.float32)
    bo = np.asarray(inputs["bo"], dtype=np.float32)

    B, S, D = x.shape
    H = 16
    HD = D // H
    G = 2
    HN = H // G
    C = HN * HD
    n_cores = B * G

    nc = _get_nc(S, D, HN, HD)
    np_md = mybir.dt.np(BF16)
    np_f8 = mybir.dt.np(FP8)
    PP = 128

    def pack_w_f8(w):
        # [K, N] -> [128, K//256, 2, N] with wf[p, t, j, n] = WSC*w[256t+128j+p, n]
        return np.ascontiguousarray(
            (w * WSC).reshape(w.shape[0] // 256, 2, PP, w.shape[1])
            .transpose(2, 0, 1, 3)
        ).astype(np_f8)

    in_maps = []
    for c in range(n_cores):
        b, g = c // G, c % G
        sl = slice(g * C, (g + 1) * C)
        xTb = np.ascontiguousarray(x[b].T)
        in_maps.append({
            "xT": xTb.astype(np_md),
            "xf": np.ascontiguousarray(
                xTb.reshape(D // PP, PP, S).transpose(1, 0, 2)).astype(np_f8),
            "wq": np.ascontiguousarray(wq[:, sl]).astype(np_md),
            "wk": np.ascontiguousarray(wk[:, sl]).astype(np_md),
            "wqf": pack_w_f8(wq[:, sl]),
            "wkf": pack_w_f8(wk[:, sl]),
            "wvf": pack_w_f8(wv[:, sl]),
            "wof": pack_w_f8(wo[sl, :]),
            "wv": np.ascontiguousarray(wv[:, sl]).astype(np_md),
            "wo": np.ascontiguousarray(wo[sl, :]).astype(np_md),
            "bq": np.ascontiguousarray(bq[sl]),
            "bk": np.ascontiguousarray(bk[sl]),
            "bv": np.ascontiguousarray(bv[sl]),
        })

    res = run_bass_kernel_spmd(nc, in_maps, core_ids=list(range(n_cores)), **spmd_kwargs)
    outs = [np.asarray(m["out"]).astype(np.float32) for m in res.results]
    out = np.stack([sum(outs[b * G + g] for g in range(G)) for b in range(B)])
    out = out + bo[None, None, :]
    return out.astype(np.float32), res

